# revision 59
# baseline (speedup 1.0000x reference)
"""Multi-head attention + residual + LayerNorm on 8 TRN2 NeuronCores.

Problem (fixed shapes): B=2, S=2048, D=1024, H=16 heads, head_dim=64.
    q,k,v = x@Wq+bq, x@Wk+bk, x@Wv+bv   (per-head split)
    probs = softmax(q@k^T/8 + mask); ctx = probs@v
    out = LayerNorm(ctx@Wo + bo + x) * gamma + beta

Stage A (tensor-parallel over heads): core c owns heads {2c, 2c+1}.
  - q/k/v projections in fp8 (e4m3) DoubleRow matmuls (256-wide contraction
    per instruction). Weights host-scaled by 8 so PSUM->SBUF copies are plain
    dtype-converting tensor_copy.
  - v produced feature-major then PE-transposed to [key, feat]; column 64 of
    v_all is ones, producing the softmax denominator for free during the ctx
    matmuls (cols 65..79 zero-pad the DoubleRow stationary to 80, 16-aligned).
  - scores: bf16, contraction 64/head; the two heads' matmuls go to disjoint
    PE row groups (tile_position (0,0)/(64,0)) into a 4-deep rotation of
    single-bank PSUM tiles, so each head's exp chain has 2 slots and the
    scores matmul hides inside the other head's exp window.
  - softmax exp: one [128,512] op per (key tile, head), engine picked by
    (jt+h)%2 so the Scalar engine (true Exp, fp8 out) and the Vector engine
    (1-instruction Schraudolph exp: int8(score/512*8*log2e + C) bit-cast as
    e4m3) run concurrently at full duty. The softmax normalization divides
    out the approximation bias; the rest is diluted ~100x by the residual.
  - ctx: fp8 DoubleRow over key-tile pairs, one block behind the scores/exp
    pipeline so v-transposes emitted as fillers stay ahead of the in-order
    PE queue; [80,512] accumulators copied out bf16 and DMA'd.
  - Emission interleaves the remaining proj chains as fillers inside the
    attention loop to keep the PE dense (PE DVFS: 512-col matmuls run 216 ns
    after ~15 us gapless, 427 ns cold; gaps reset the ramp).
Stage B (data-parallel over rows): core c owns rows [512c, 512(c+1)).
  - Host folds the softmax denominator into ctxn (one fp8 tensor), then
    ctxn@Wo in fp8 DoubleRow, +residual, LayerNorm (bn_stats on DVE, final
    affine via Scalar Identity activation).
Host only reshuffles/casts arrays between stages (plus 65k reciprocals,
the denominator fold and the x+bo add, all O(R*D) ~ small vs attention).
"""

import numpy as np
import ml_dtypes

import concourse.bacc as bacc
import concourse.bass as bass
import concourse.tile as tile
from concourse import mybir
from concourse.bass_utils import run_bass_kernel_spmd

BF16 = ml_dtypes.bfloat16
E4M3 = ml_dtypes.float8_e4m3

B, S, D, H = 2, 2048, 1024, 16
HD = D // H          # 64
NCORES = 8
HPC = H // NCORES    # 2 heads per core
R = B * S            # 4096 rows
RPC = R // NCORES    # 512 rows per core in stage B
LN_EPS = 1e-12

LOG2E = 1.4426950408889634
SCH_C = 0.395            # Schraudolph constant tuned for RNE-int8+e4m3
SCH_MUL = 8 * LOG2E / 512
SCH_ADD = 56.0 + 8 * SCH_C
# jt indices handled by the Scalar engine's true Exp (rest: DVE Schraudolph)
ACT_JT = frozenset({0, 2, 4, 6, 8, 10, 12, 14, 15})

_cache = {}
PROFILE = False
last_exec_ns = {}


def _build_stage_a(qkv_trivial=True):
    nc = bacc.Bacc("TRN2", target_bir_lowering=False, debug=False,
                   num_devices=NCORES)
    f32 = mybir.dt.float32
    bf16 = mybir.dt.bfloat16
    fp8 = mybir.dt.float8e4
    DR = mybir.MatmulPerfMode.DoubleRow

    xts = nc.dram_tensor("xts", [8, 128, 4, 2, 512], fp8,
                         kind="ExternalInput").ap()
    wkq = nc.dram_tensor("wkq", [2, 128, 4, 2, 128], fp8,
                         kind="ExternalInput").ap()
    wv = nc.dram_tensor("wv", [128, 4, 2, 128], fp8,
                        kind="ExternalInput").ap()
    bqk = nc.dram_tensor("bqk", [128, 2], f32, kind="ExternalInput").ap()
    bvf = nc.dram_tensor("bvf", [128, 2, 64], f32,
                         kind="ExternalInput").ap()
    mask_e = nc.dram_tensor("mask_e", [128, B, 16], f32,
                            kind="ExternalInput").ap()
    mask_s = nc.dram_tensor("mask_s", [128, B, 16], f32,
                            kind="ExternalInput").ap()
    id8 = nc.dram_tensor("id8", [128, 128], bf16, kind="ExternalInput").ap()
    ctxT = nc.dram_tensor("ctxT", [HPC, 8, 65, 512], bf16,
                          kind="ExternalOutput").ap()

    with tile.TileContext(nc) as tc:
        with (
            tc.tile_pool(name="singles", bufs=1) as singles,
            tc.tile_pool(name="xt", bufs=8) as xt_pool,
            tc.tile_pool(name="vt", bufs=3) as vt_pool,
            tc.tile_pool(name="pps", bufs=1, space="PSUM") as proj_ps,
            tc.tile_pool(name="ex", bufs=18) as ex_pool,
            tc.tile_pool(name="ob", bufs=3) as ob_pool,
        ):
            # k weights land first (first chain), then q, then v
            w_kq = singles.tile([128, 2, 4, 2, 128], fp8)
            nc.scalar.dma_start(out=w_kq[:, 0], in_=wkq[0])
            nc.scalar.dma_start(out=w_kq[:, 1], in_=wkq[1])
            w_v = singles.tile([128, 4, 2, 128], fp8)
            nc.scalar.dma_start(out=w_v, in_=wv[:])
            bqk_sb = singles.tile([128, 2], f32)
            bv_sb = singles.tile([128, 2, 64], f32)
            me_sb = singles.tile([128, B, 16], f32)
            ms_sb = singles.tile([128, B, 16], f32)
            id_sb = singles.tile([128, 128], bf16)

            q_sb = singles.tile([128, R], bf16)
            k_sb = singles.tile([128, R], bf16)
            # [key_part, b, key-tile-pair, pair-slab, h*80+feat]
            # only the ones/pad columns need init; the 0:64 data regions
            # are fully overwritten by the v copies before any ctx read
            v_all = singles.tile([128, B, 8, 2, 160], fp8)
            nc.vector.memset(v_all[:, :, :, :, 64:80], 0.0)
            nc.vector.memset(v_all[:, :, :, :, 144:160], 0.0)
            nc.vector.memset(v_all[:, :, :, :, 64:65], 1.0)
            nc.vector.memset(v_all[:, :, :, :, 144:145], 1.0)

            cp_eng = [0]

            def cp(out, in_):
                eng = cp_eng[0] % 2
                cp_eng[0] += 1
                if eng == 0:
                    nc.scalar.copy(out=out, in_=in_)
                else:
                    nc.vector.tensor_copy(out=out, in_=in_)

            def xt_load(rb):
                xt = xt_pool.tile([128, 4, 2, 512], fp8, tag="xt",
                                  name=f"xt_{rb}")
                nc.sync.dma_start(out=xt[:, 0:2], in_=xts[rb, :, 0:2])
                nc.gpsimd.dma_start(out=xt[:, 2:4], in_=xts[rb, :, 2:4])
                return xt

            def proj_piece(xt, rb, kq, dst_sb, bias_col):
                def _p():
                    ps = proj_ps.tile([128, 512], f32, tag="prj", bufs=2,
                                      name=f"pj_{rb}_{kq}")
                    for k2 in range(4):
                        nc.tensor.matmul(
                            ps,
                            lhsT=w_kq[:, kq, k2],
                            rhs=xt[:, k2],
                            start=(k2 == 0), stop=(k2 == 3),
                            perf_mode=DR)
                    dst = dst_sb[:, rb * 512:(rb + 1) * 512]
                    if qkv_trivial:
                        cp(dst, ps)
                    else:
                        nc.vector.tensor_scalar(
                            out=dst, in0=ps, scalar1=1.0,
                            scalar2=bqk_sb[:, bias_col:bias_col + 1],
                            op0=mybir.AluOpType.mult,
                            op1=mybir.AluOpType.add)
                return _p

            def v_pieces(xt, rb):
                # [keys, feat] v via feature-major projection + PE transpose
                b = rb // 4
                vt = vt_pool.tile([128, 512], bf16, tag="vt",
                                  name=f"vt_{rb}")

                def vproj():
                    ps = proj_ps.tile([128, 512], f32, tag="prj", bufs=2,
                                      name=f"pv_{rb}")
                    for k2 in range(4):
                        nc.tensor.matmul(
                            ps, lhsT=w_v[:, k2], rhs=xt[:, k2],
                            start=(k2 == 0), stop=(k2 == 3), perf_mode=DR)
                    if qkv_trivial:
                        cp(vt, ps)
                    else:
                        nc.vector.tensor_tensor(
                            out=vt, in0=ps, in1=bv_sb,
                            op=mybir.AluOpType.add)

                def vtrans(rt):
                    def _t():
                        trp = proj_ps.tile([128, 128], bf16, tag="prj",
                                           bufs=2, name=f"tr_{rb}_{rt}")
                        nc.tensor.transpose(
                            trp, vt[:, rt * 128:(rt + 1) * 128], id_sb)
                        jt = (rb % 4) * 4 + rt
                        dst = v_all[:, b, jt // 2, jt % 2].rearrange(
                            "p (h c) -> p h c", h=2)[:, :, 0:64]
                        src = trp.rearrange("p (h c) -> p h c", h=2)
                        cp(dst, src)
                    return _t

                return [vproj, vtrans(0), vtrans(1), vtrans(2), vtrans(3)]

            def emit_ctx_prev(cps, prev, jtp):
                pb = prev[0]
                for h in range(HPC):
                    nc.tensor.matmul(
                        cps[h],
                        lhsT=v_all[:, pb, jtp, :, h * 80:(h + 1) * 80],
                        rhs=prev[2][jtp][:, :, h],
                        start=(jtp == 0), stop=(jtp == 7),
                        perf_mode=DR)

            def dma_ctx(cps, b, ib):
                blk = b * 4 + ib
                for h in range(HPC):
                    ob = ob_pool.tile([65, 512], bf16, tag="ob")
                    if h == 0:
                        nc.scalar.copy(out=ob, in_=cps[h][0:65, :])
                    else:
                        nc.vector.tensor_copy(out=ob, in_=cps[h][0:65, :])
                    eng = nc.sync if h == 0 else nc.gpsimd
                    eng.dma_start(out=ctxT[h, blk], in_=ob)

            def attention_block(b, ib, prev, fillers, act9):
                # Emits scores+exp for (b, ib) and the ctx matmuls for the
                # previous block (software pipeline, one block behind) so
                # v-transposes emitted as fillers here stay ahead of the
                # ctx that consumes them on the in-order PE queue.
                q_lo = b * S + ib * 512
                fillers = list(fillers)
                exs = []
                cps = None
                if prev is not None:
                    cps = [proj_ps.tile([80, 512], f32, tag=f"ctx{h}",
                                        bufs=1,
                                        name=f"ctx_{prev[0]}_{prev[1]}_{h}")
                           for h in range(HPC)]
                for jtp in range(8):
                    ex = ex_pool.tile([128, 2, 2, 512], fp8, tag="ex",
                                      name=f"ex_{b}_{ib}_{jtp}")
                    for sl in range(2):
                        jt = jtp * 2 + sl
                        if fillers:
                            fillers.pop(0)()
                        k_lo = b * S + jt * 128
                        for h in range(2):
                            sp = proj_ps.tile(
                                [128, 512], f32, tag="scp", bufs=4,
                                name=f"sp_{b}_{ib}_{jt}_{h}")
                            nc.tensor.matmul(
                                sp,
                                lhsT=k_sb[64 * h:64 * h + 64,
                                          k_lo:k_lo + 128],
                                rhs=q_sb[64 * h:64 * h + 64,
                                         q_lo:q_lo + 512],
                                start=True, stop=True,
                                tile_position=(64 * h, 0))
                            if (jt + h) % 2 == 0:
                                nc.scalar.activation(
                                    out=ex[:, sl, h], in_=sp,
                                    func=mybir.ActivationFunctionType.Exp,
                                    bias=me_sb[:, b, jt:jt + 1],
                                    scale=1.0 / 512)
                            else:
                                nc.vector.tensor_scalar(
                                    out=ex[:, sl, h].bitcast(
                                        mybir.dt.int8),
                                    in0=sp, scalar1=SCH_MUL,
                                    scalar2=ms_sb[:, b, jt:jt + 1],
                                    op0=mybir.AluOpType.mult,
                                    op1=mybir.AluOpType.add)
                    exs.append(ex)
                    if prev is not None:
                        emit_ctx_prev(cps, prev, jtp)
                if prev is not None:
                    dma_ctx(cps, prev[0], prev[1])
                return exs

            # --- emission schedule -------------------------------------
            xts_sb = [xt_load(rb) for rb in range(8)]
            # small inputs issued after the x tiles, off the scalar queue,
            # so the first PSUM copies aren't stuck behind their issue cost
            nc.gpsimd.dma_start(out=bqk_sb, in_=bqk[:])
            nc.gpsimd.dma_start(out=bv_sb, in_=bvf[:])
            nc.gpsimd.dma_start(out=me_sb, in_=mask_e[:])
            nc.gpsimd.dma_start(out=ms_sb, in_=mask_s[:])
            nc.gpsimd.dma_start(out=id_sb, in_=id8[:])

            def q_piece(rb):
                return proj_piece(xts_sb[rb], rb, 1, q_sb, 0)

            def k_piece(rb):
                return proj_piece(xts_sb[rb], rb, 0, k_sb, 1)

            # lead-in: just k and q for the first 512 rows of batch 0
            k_piece(0)()
            q_piece(0)()

            vp = [v_pieces(xts_sb[rb], rb) for rb in range(8)]
            fills = {
                (0, 0): ([k_piece(1), k_piece(2), k_piece(3),
                          q_piece(1)] + vp[0] + vp[1]),
                (0, 1): ([q_piece(2)] + vp[2]
                         + [k_piece(4), k_piece(5), k_piece(6),
                            k_piece(7)]),
                (0, 2): [q_piece(3)] + vp[3] + [vp[4][0]],
                (0, 3): [q_piece(4)] + vp[4][1:] + vp[5],
                (1, 0): [q_piece(5)] + vp[6] + [vp[7][0]],
                (1, 1): [q_piece(6)] + vp[7][1:],
                (1, 2): [q_piece(7)],
                (1, 3): [],
            }

            blocks = [(0, 0), (0, 1), (0, 2), (0, 3),
                      (1, 0), (1, 1), (1, 2), (1, 3)]
            prev = None
            for i, (b, ib) in enumerate(blocks):
                exs = attention_block(b, ib, prev, fills[(b, ib)],
                                      act9=(i % 2 == 0))
                prev = (b, ib, exs)
            # flush the last block's ctx
            cps = [proj_ps.tile([80, 512], f32, tag=f"ctx{h}", bufs=1,
                                name=f"ctx_1_3_{h}")
                   for h in range(HPC)]
            for jtp in range(8):
                emit_ctx_prev(cps, prev, jtp)
            dma_ctx(cps, 1, 3)

    nc.compile()
    return nc


def _build_stage_b(ln_trivial=True):
    nc = bacc.Bacc("TRN2", target_bir_lowering=False, debug=False,
                   num_devices=NCORES)
    f32 = mybir.dt.float32
    bf16 = mybir.dt.bfloat16
    fp8 = mybir.dt.float8e4
    DR = mybir.MatmulPerfMode.DoubleRow

    ctxn = nc.dram_tensor("ctxn", [128, 4, 2, 512], fp8,
                          kind="ExternalInput").ap()
    wo = nc.dram_tensor("wo", [128, 2, 4, 2, 512], fp8,
                        kind="ExternalInput").ap()
    xpb = nc.dram_tensor("xpb", [RPC, D], bf16, kind="ExternalInput").ap()
    gamma = nc.dram_tensor("gamma", [D], f32, kind="ExternalInput").ap()
    beta = nc.dram_tensor("beta", [D], f32, kind="ExternalInput").ap()
    out = nc.dram_tensor("out", [RPC, D], bf16, kind="ExternalOutput").ap()

    with tile.TileContext(nc) as tc:
        with (
            tc.tile_pool(name="singles", bufs=1) as singles,
            tc.tile_pool(name="xp", bufs=4) as xp_pool,
            tc.tile_pool(name="hid", bufs=2) as h_pool,
            tc.tile_pool(name="ps", bufs=4, space="PSUM") as ps_pool,
            tc.tile_pool(name="stat", bufs=4) as stat_pool,
            tc.tile_pool(name="outp", bufs=2) as out_pool,
        ):
            engs = [nc.sync, nc.scalar, nc.gpsimd]
            wo_sb = singles.tile([128, 2, 4, 2, 512], fp8)
            ctx_sb = singles.tile([128, 4, 2, 512], fp8)
            # 3-queue parallel staging, first-needed chunks first;
            # every DMA source is contiguous 2KB+ per partition
            nc.sync.dma_start(out=ctx_sb[:, 0], in_=ctxn[:, 0])
            nc.scalar.dma_start(out=wo_sb[:, 0, 0], in_=wo[:, 0, 0])
            nc.gpsimd.dma_start(out=wo_sb[:, 1, 0], in_=wo[:, 1, 0])
            nc.sync.dma_start(out=ctx_sb[:, 1], in_=ctxn[:, 1])
            nc.scalar.dma_start(out=wo_sb[:, 0, 1], in_=wo[:, 0, 1])
            nc.gpsimd.dma_start(out=wo_sb[:, 1, 1], in_=wo[:, 1, 1])
            nc.sync.dma_start(out=ctx_sb[:, 2:4], in_=ctxn[:, 2:4])
            nc.scalar.dma_start(out=wo_sb[:, 0, 2:4], in_=wo[:, 0, 2:4])
            nc.gpsimd.dma_start(out=wo_sb[:, 1, 2:4], in_=wo[:, 1, 2:4])
            MT = RPC // 128  # 4 row tiles
            xps = []
            for mt in range(MT):
                xp = xp_pool.tile([128, D], bf16, name=f"xp_{mt}")
                engs[mt % 3].dma_start(
                    out=xp, in_=xpb[mt * 128:(mt + 1) * 128, :])
                xps.append(xp)
            if not ln_trivial:
                gm_sb = singles.tile([128, D], f32)
                nc.scalar.dma_start(
                    out=gm_sb,
                    in_=bass.AP(tensor=gamma.tensor, offset=gamma.offset,
                                ap=[[0, 128]] + list(gamma.ap)))
                bt_sb = singles.tile([128, D], f32)
                nc.scalar.dma_start(
                    out=bt_sb,
                    in_=bass.AP(tensor=beta.tensor, offset=beta.offset,
                                ap=[[0, 128]] + list(beta.ap)))
            eps_sb = singles.tile([128, 1], f32)
            nc.vector.memset(eps_sb, LN_EPS)

            for mt in range(MT):
                xp = xps[mt]
                hid = h_pool.tile([128, D], f32)
                for nb in range(2):
                    ps = ps_pool.tile([128, 512], f32)
                    for k2 in range(4):
                        nc.tensor.matmul(
                            ps,
                            lhsT=ctx_sb[:, k2, :, mt * 128:(mt + 1) * 128],
                            rhs=wo_sb[:, nb, k2],
                            start=(k2 == 0), stop=(k2 == 3), perf_mode=DR)
                    nc.vector.scalar_tensor_tensor(
                        out=hid[:, nb * 512:(nb + 1) * 512], in0=ps,
                        scalar=1.0 / 64,
                        in1=xp[:, nb * 512:(nb + 1) * 512],
                        op0=mybir.AluOpType.mult,
                        op1=mybir.AluOpType.add)
                st = stat_pool.tile([128, 2, 6], f32, tag="st")
                for g in range(2):
                    nc.vector.bn_stats(out=st[:, g, :],
                                       in_=hid[:, g * 512:(g + 1) * 512])
                mv = stat_pool.tile([128, 2], f32, tag="mv")
                nc.vector.bn_aggr(out=mv, in_=st)
                sd = stat_pool.tile([128, 1], f32, tag="sd")
                nc.scalar.activation(out=sd, in_=mv[:, 1:2],
                                     func=mybir.ActivationFunctionType.Sqrt,
                                     bias=eps_sb, scale=1.0)
                rs = stat_pool.tile([128, 1], f32, tag="rs")
                nc.vector.reciprocal(out=rs, in_=sd)
                nmr = stat_pool.tile([128, 1], f32, tag="nmr")
                nc.vector.tensor_scalar(out=nmr, in0=mv[:, 0:1],
                                        scalar1=rs, scalar2=-1.0,
                                        op0=mybir.AluOpType.mult,
                                        op1=mybir.AluOpType.mult)
                ot = out_pool.tile([128, D], bf16)
                nc.scalar.activation(
                    out=ot, in_=hid,
                    func=mybir.ActivationFunctionType.Identity,
                    bias=nmr, scale=rs)
                if not ln_trivial:
                    nc.vector.tensor_mul(out=ot, in0=ot, in1=gm_sb)
                    nc.vector.tensor_add(out=ot, in0=ot, in1=bt_sb)
                engs[mt % 2].dma_start(
                    out=out[mt * 128:(mt + 1) * 128, :], in_=ot)

    nc.compile()
    return nc


def _get(name, **kw):
    key = (name, tuple(sorted(kw.items())))
    if key not in _cache:
        _cache[key] = (_build_stage_a(**kw) if name == "a"
                       else _build_stage_b(**kw))
    return _cache[key]


def _run(nc, in_maps, label):
    kwargs = {}
    if PROFILE:
        kwargs = dict(trace=True)
    res = run_bass_kernel_spmd(nc, in_maps, list(range(NCORES)), **kwargs)
    if PROFILE:
        last_exec_ns[label] = res.exec_time_ns
    return res.results


def kernel(**inputs):
    x = np.asarray(inputs["input_tensor"], dtype=np.float32)
    mask = np.asarray(inputs["attention_mask"], dtype=np.float32)[:, 0, 0, :]
    Wq = np.asarray(inputs["Wq"], dtype=np.float32)
    bq = np.asarray(inputs["bq"], dtype=np.float32)
    Wk = np.asarray(inputs["Wk"], dtype=np.float32)
    bk = np.asarray(inputs["bk"], dtype=np.float32)
    Wv = np.asarray(inputs["Wv"], dtype=np.float32)
    bv = np.asarray(inputs["bv"], dtype=np.float32)
    Wo = np.asarray(inputs["Wo"], dtype=np.float32)
    bo = np.asarray(inputs["bo"], dtype=np.float32)
    gamma = np.asarray(inputs["ln_gamma"], dtype=np.float32)
    beta = np.asarray(inputs["ln_beta"], dtype=np.float32)

    qkv_trivial = bool(np.all(bq == 0) and np.all(bk == 0)
                       and np.all(bv == 0))

    xf = x.reshape(R, D)
    # [rb, p, k2, slab, row]: element = xf[rb*512+row, k2*256+slab*128+p]
    xts8 = np.ascontiguousarray(
        xf.reshape(8, 512, 4, 2, 128).transpose(0, 4, 2, 3, 1)).astype(E4M3)
    mask_h = np.ascontiguousarray(
        mask.reshape(B, 16, 128).transpose(2, 0, 1))
    mask_sch = (SCH_ADD + 8 * LOG2E * mask_h).astype(np.float32)
    id8_h = np.eye(128, dtype=BF16)

    def wprep(Wc):  # [1024, ncol] -> [128, 4, 2, ncol] fp8, x8 scale
        ncol = Wc.shape[1]
        return np.ascontiguousarray(
            (8 * Wc).reshape(4, 2, 128, ncol).transpose(2, 0, 1, 3)
        ).astype(E4M3)

    in_maps_a = []
    for c in range(NCORES):
        cs = slice(c * 128, (c + 1) * 128)
        in_maps_a.append({
            "xts": xts8,
            "wkq": np.ascontiguousarray(
                np.stack([wprep(Wk[:, cs]), wprep(Wq[:, cs])], axis=0)),
            "wv": wprep(Wv[:, cs]),
            "bqk": np.ascontiguousarray(
                8 * np.stack([bq[cs], bk[cs]], axis=1)).astype(np.float32),
            "bvf": np.ascontiguousarray(np.broadcast_to(
                (8 * bv[cs]).reshape(1, 2, 64),
                (128, 2, 64))).astype(np.float32),
            "mask_e": mask_h,
            "mask_s": mask_sch,
            "id8": id8_h,
        })
    res_a = _run(_get("a", qkv_trivial=qkv_trivial), in_maps_a, "stage_a")

    # Assemble normalized ctx: ct rows are 8*U (U = sum p~ v), row 64 is
    # den = sum p~.  ctxn = 4 * ctx_true = ct[0:64] * (0.5 / den).
    ctxn_full = np.empty((D, R), dtype=np.float32)
    for c in range(NCORES):
        ct = np.asarray(res_a[c]["ctxT"], dtype=np.float32)
        ct = ct.transpose(0, 2, 1, 3).reshape(HPC, 65, R)
        for h in range(HPC):
            rows = slice((2 * c + h) * 64, (2 * c + h + 1) * 64)
            ctxn_full[rows] = ct[h, 0:64] * (0.5 / ct[h, 64])
    ctxn8_full = ctxn_full.astype(E4M3)

    wo_b = np.ascontiguousarray(
        (16 * Wo).reshape(4, 2, 128, 2, 512).transpose(
            2, 3, 0, 1, 4)).astype(E4M3)
    xpb_f = (xf + bo[None, :]).astype(BF16)
    ln_trivial = bool(np.all(gamma == 1.0) and np.all(beta == 0.0))

    def bprep(a, rs):  # [1024, R] -> [128, 4, 2, 512] core slice
        return np.ascontiguousarray(
            a[:, rs].reshape(4, 2, 128, RPC).transpose(2, 0, 1, 3))

    in_maps_b = []
    for c in range(NCORES):
        rs = slice(c * RPC, (c + 1) * RPC)
        in_maps_b.append({
            "ctxn": bprep(ctxn8_full, rs),
            "wo": wo_b,
            "xpb": np.ascontiguousarray(xpb_f[rs]),
            "gamma": gamma,
            "beta": beta,
        })
    res_b = _run(_get("b", ln_trivial=ln_trivial), in_maps_b, "stage_b")

    out = np.concatenate(
        [np.asarray(res_b[c]["out"], dtype=np.float32)
         for c in range(NCORES)], axis=0)
    return out.reshape(B, S, D)
